# revision 23
# baseline (speedup 1.0000x reference)
"""Multi-head causal attention on 8 Trainium2 NeuronCores (Bass/Tile).

Problem: B=4, S=1024, D=1024, H=16 heads (dk=64), causal mask, fp32 I/O.

Sharding: 8 cores = 4 batches x 2 head-groups (8 heads each).
  Wq/Wk/Wv sharded column-wise by head (tensor parallel), Wo row-wise;
  the Wo all-reduce is a host-side pairwise sum (2 cores per batch).

Per-core kernel (bf16 matmul operands, fp32 PSUM accumulate, ~4.7e-3
absmax-relative vs the fp32 reference):
  phase P: Q^T (zero-padded per head: slot h holds Q_h^T on its 64
           partitions, zeros elsewhere, so score matmuls contract K=128
           at full rate against the packed K^T without mixing heads),
           K^T packed [128, 4, S], V -> v_sb [128, ki, head, 65] with a
           65th ones column per head (softmax denominator trick).
           PSUM->SBUF copies run on the otherwise-idle ACT engine.
  phase A: per head-chunk hc and q-half qj: scores^T [k=128, q<=512]
           (causally width-trimmed), exp on ACT (no max subtraction:
           |scores/8| < ~6), 0/1 mask multiply only on the diagonal
           128-block, attnV accumulated over k-chunks with lhsT =
           V_ext [k, 65]; row 64 = denominator. Denominator rows are
           copied on ACT and DMA-gathered into den8 (keeps the DVE FIFO
           and PE stream free of per-pair round-trips).
  phase O: two batched reciprocals, selector-matmul broadcast of 1/den
           over partition halves, in-place normalize of headout^T,
           output projection accumulating over d-chunks, DMA out.
"""

from contextlib import ExitStack

import ml_dtypes
import numpy as np

import concourse.bacc as bacc
import concourse.tile as tile
from concourse import mybir
from concourse.bass_utils import run_bass_kernel_spmd

A_SUB = {"mask", "av", "extract", "houtcp", "recip"}  # phase-A bisection knob

F32R = mybir.dt.float32r
F32 = mybir.dt.float32
BF16 = mybir.dt.bfloat16
EXP = mybir.ActivationFunctionType.Exp

S = 1024  # sequence length
D = 1024  # model dim
DK = 64  # head dim
HPC = 8  # heads per core
N_CORES = 8
SCALE = 1.0 / np.sqrt(DK)  # folded into the exp activation


def _emit(nc, tc, t, rep, phases=("P", "A", "O")):
    """Emit one full forward pass. `t` = dict of dram tensors."""
    ctx = ExitStack()
    with ctx:
        # ---- long-lived SBUF (per repeat; pools free at phase end) ----
        main = ctx.enter_context(tc.tile_pool(name=f"main{rep}", bufs=1))

        # Q^T zero-padded per head: slot h holds Q_h^T on its 64 partitions,
        # zeros on the other 64 -> score matmuls contract K=128 (full rate)
        # against the packed kt_sb without mixing heads.
        qtz = main.tile([128, 8, S], BF16)
        kt_sb = main.tile([128, 4, S], BF16)
        v_sb = main.tile([128, 8, 8, 65], BF16)  # s-part: (ki, head, d+1)
        hout_sb = main.tile([128, 4, S], BF16)  # headout^T (unnormalized)
        maskd = main.tile([128, 128], BF16)  # diagonal-block 0/1 mask
        sel8 = main.tile([8, 512], F32R)
        den8 = main.tile([8, S], F32)
        rec8 = main.tile([8, S], F32R)
        wo_sb = main.tile([128, 4, S], BF16)

        nc.sync.dma_start(out=sel8, in_=t["sel8"][:, :])
        nc.sync.dma_start(out=maskd, in_=t["maskd"][:, :])
        nc.sync.dma_start(
            out=v_sb.rearrange("p a b c -> p (a b) c")[:, :, 64:65],
            in_=t["ones_col"][:, :, None],
        )
        nc.vector.memset(qtz.rearrange("p a b -> p (a b)"), 0.0)

        # ================= phase P: projections =================
        if "P" in phases:
         with (
            tc.tile_pool(name=f"xin{rep}", bufs=2) as xpool,
            tc.tile_pool(name=f"win{rep}", bufs=2) as wpool,
            tc.tile_pool(name=f"pps{rep}", bufs=2, space="PSUM") as ppool,
        ):
            for which, xname, wname in (
                ("q", "xq_t", "wq_t"),
                ("k", "xk_t", "wk_t"),
                ("v", "xv_t", "wv_t"),
            ):
                x_sb = xpool.tile([128, 8, S], BF16, tag="x")
                w_sb = wpool.tile([128, 8, 512], BF16, tag="w")
                xdr = t[xname].rearrange("(n p) s -> p n s", p=128)
                nc.sync.dma_start(
                    out=w_sb.rearrange("p a b -> p (a b)"), in_=t[wname][:, :]
                )
                for half in range(2):  # column halves: s-half chains start early
                    nc.sync.dma_start(
                        out=x_sb[:, :, 512 * half : 512 * (half + 1)],
                        in_=xdr[:, :, 512 * half : 512 * (half + 1)],
                    )
                if which in ("q", "k"):
                    for sj in range(2):
                        for dtile in range(4):
                            ps = ppool.tile([128, 512], F32, tag="ps")
                            for c in range(8):
                                nc.tensor.matmul(
                                    ps,
                                    w_sb[:, c, 128 * dtile : 128 * (dtile + 1)],
                                    x_sb[:, c, 512 * sj : 512 * (sj + 1)],
                                    start=(c == 0),
                                    stop=(c == 7),
                                )
                            sjs = slice(512 * sj, 512 * (sj + 1))
                            if which == "q":
                                nc.scalar.copy(
                                    qtz[0:64, 2 * dtile, sjs], ps[0:64, :]
                                )
                                nc.scalar.copy(
                                    qtz[64:128, 2 * dtile + 1, sjs], ps[64:128, :]
                                )
                            else:
                                nc.scalar.copy(kt_sb[:, dtile, sjs], ps)
                else:
                    for stile in range(8):
                        ps = ppool.tile([128, 512], F32, tag="ps")
                        for c in range(8):
                            nc.tensor.matmul(
                                ps,
                                x_sb[:, c, 128 * stile : 128 * (stile + 1)],
                                w_sb[:, c, :],
                                start=(c == 0),
                                stop=(c == 7),
                            )
                        nc.scalar.copy(
                            v_sb[:, stile, :, 0:64],
                            ps.rearrange("p (h c) -> p h c", c=64),
                        )

        # ========= phase A + O fused: per q-half, attention then outproj ====
        if "A" in phases:
         with (
            tc.tile_pool(name=f"scps{rep}", bufs=2, space="PSUM") as scpool,
            tc.tile_pool(name=f"avps{rep}", bufs=2, space="PSUM") as avpool,
            tc.tile_pool(name=f"ops{rep}", bufs=2, space="PSUM") as opool,
            tc.tile_pool(name=f"epool{rep}", bufs=12) as epool,
            tc.tile_pool(name=f"xtr{rep}", bufs=6) as xtr,
            tc.tile_pool(name=f"osb{rep}", bufs=3) as osb,
        ):
            if "O" in phases:  # prefetch Wo during attention
                nc.sync.dma_start(
                    out=wo_sb.rearrange("p a b -> p (a b)"), in_=t["wo_s"][:, :]
                )
            for qj in range(2):
                qsl = slice(512 * qj, 512 * (qj + 1))
                for hc in range(4):
                    kmax = 4 if qj == 0 else 8
                    o_e = avpool.tile([128, 512], F32, tag="av")
                    o_o = avpool.tile([128, 512], F32, tag="av")
                    for ki in range(kmax):
                        # causal: columns [0, b) of this 512-block are fully
                        # masked for this k-chunk; diagonal block at [b, b+128)
                        b = 128 * max(0, ki - 4 * qj)
                        kis = slice(128 * ki, 128 * (ki + 1))
                        sc = scpool.tile([128, 2, 512], F32, tag="sc")
                        nc.tensor.matmul(
                            sc[:, 0, b:512],
                            kt_sb[:, hc, kis],
                            qtz[:, 2 * hc, 512 * qj + b : 512 * (qj + 1)],
                            start=True,
                            stop=True,
                        )
                        nc.tensor.matmul(
                            sc[:, 1, b:512],
                            kt_sb[:, hc, kis],
                            qtz[:, 2 * hc + 1, 512 * qj + b : 512 * (qj + 1)],
                            start=True,
                            stop=True,
                        )
                        ee = epool.tile([128, 2, 512], BF16, tag="e")
                        nc.scalar.activation(
                            ee[:, :, b:512],
                            sc[:, :, b:512],
                            EXP,
                            scale=float(SCALE),
                        )
                        if ki - 4 * qj >= 0:  # diagonal block: 0/1 mask
                            nc.vector.tensor_mul(
                                ee[:, 0, b : b + 128], ee[:, 0, b : b + 128], maskd
                            )
                            nc.vector.tensor_mul(
                                ee[:, 1, b : b + 128], ee[:, 1, b : b + 128], maskd
                            )
                        nc.tensor.matmul(
                            o_e[0:65, b:512],
                            v_sb[:, ki, 2 * hc, :],
                            ee[:, 0, b:512],
                            start=(ki == 0),
                            stop=(ki == kmax - 1),
                        )
                        nc.tensor.matmul(
                            o_o[0:65, b:512],
                            v_sb[:, ki, 2 * hc + 1, :],
                            ee[:, 1, b:512],
                            start=(ki == 0),
                            stop=(ki == kmax - 1),
                        )
                    # extract headout^T + denominator rows
                    nc.vector.tensor_copy(hout_sb[0:64, hc, qsl], o_e[0:64, :])
                    otmp = xtr.tile([64, 512], BF16, tag="otmp")
                    nc.vector.tensor_copy(otmp, o_o[0:64, :])
                    nc.sync.dma_start(out=hout_sb[64:128, hc, qsl], in_=otmp)
                    de_t = xtr.tile([1, 512], F32, tag="de")
                    do_t = xtr.tile([1, 512], F32, tag="do")
                    nc.vector.tensor_copy(de_t, o_e[64:65, :])
                    nc.vector.tensor_copy(do_t, o_o[64:65, :])
                    nc.sync.dma_start(out=den8[2 * hc : 2 * hc + 1, qsl], in_=de_t)
                    nc.sync.dma_start(
                        out=den8[2 * hc + 1 : 2 * hc + 2, qsl], in_=do_t
                    )
            for qj in range(2) if "O" in phases else []:
                qsl = slice(512 * qj, 512 * (qj + 1))
                # ---- normalize + output projection for this q-half
                with nc.allow_low_precision(reason="softmax reciprocal"):
                    nc.vector.reciprocal(rec8[:, qsl], den8[:, qsl])
                for hc in range(4):
                    bp = opool.tile([128, 512], F32, tag="op")
                    nc.tensor.matmul(
                        bp,
                        sel8[:, 128 * hc : 128 * (hc + 1)],
                        rec8[:, qsl],
                        start=True,
                        stop=True,
                    )
                    nc.vector.tensor_mul(
                        hout_sb[:, hc, qsl], hout_sb[:, hc, qsl], bp
                    )
                for stile in range(4 * qj, 4 * qj + 4):
                    out_sb = osb.tile([128, S], F32, tag="out")
                    for ej in range(2):
                        fp = opool.tile([128, 512], F32, tag="op")
                        for hc in range(4):
                            nc.tensor.matmul(
                                fp,
                                hout_sb[:, hc, 128 * stile : 128 * (stile + 1)],
                                wo_sb[:, hc, 512 * ej : 512 * (ej + 1)],
                                start=(hc == 0),
                                stop=(hc == 3),
                            )
                        esl = slice(512 * ej, 512 * (ej + 1))
                        if ej == 0:
                            nc.vector.tensor_copy(out_sb[:, esl], fp)
                        else:
                            nc.scalar.copy(out_sb[:, esl], fp)
                    nc.sync.dma_start(
                        out=t["out_p"][128 * stile : 128 * (stile + 1), :],
                        in_=out_sb,
                    )


def _build_phases(phases, repeat=1):
    return _build(repeat, phases=phases)


def _build(repeat=1, phases=("P", "A", "O")):
    nc = bacc.Bacc()
    t = {}
    for name in ("xq_t", "xk_t", "xv_t"):
        t[name] = nc.dram_tensor(name, [D, S], BF16, kind="ExternalInput")
    for name in ("wq_t", "wk_t", "wv_t"):
        t[name] = nc.dram_tensor(name, [128, 8 * 512], BF16, kind="ExternalInput")
    t["wo_s"] = nc.dram_tensor("wo_s", [128, 4 * D], BF16, kind="ExternalInput")
    t["maskd"] = nc.dram_tensor("maskd", [128, 128], BF16, kind="ExternalInput")
    t["sel8"] = nc.dram_tensor("sel8", [8, 512], F32R, kind="ExternalInput")
    t["ones_col"] = nc.dram_tensor("ones_col", [128, 64], BF16, kind="ExternalInput")
    t["out_p"] = nc.dram_tensor("out_p", [S, D], F32, kind="ExternalOutput")

    with tile.TileContext(nc) as tc:
        if repeat == 1:
            _emit(nc, tc, t, 0, phases)
        else:
            with tc.For_i(0, repeat, 1):
                _emit(nc, tc, t, 0, phases)
    nc.compile()
    return nc


_CACHE = {}


def _get(repeat=1):
    if repeat not in _CACHE:
        _CACHE[repeat] = _build(repeat)
    return _CACHE[repeat]


def _host_prep(query, key, value, mask, Wq, Wk, Wv, Wo):
    """Build the per-core in_maps. Returns None if mask isn't causal tril."""
    m = np.asarray(mask)[0, 0]
    if not np.array_equal(m, np.tril(np.ones((S, S), m.dtype))):
        return None

    bf = ml_dtypes.bfloat16

    # diagonal-block mask (same for every diagonal tile under causal tril)
    maskd = m[0:128, 0:128].T.astype(bf)

    sel8 = np.zeros((8, 512), np.float32)
    for hc in range(4):
        sel8[2 * hc, 128 * hc : 128 * hc + 64] = 1.0
        sel8[2 * hc + 1, 128 * hc + 64 : 128 * hc + 128] = 1.0
    ones_col = np.ones((128, 64), bf)

    def ileave(a):  # [R, C] -> [128, (R//128)*C]: chunk-c data contiguous per p
        R, C = a.shape
        return np.ascontiguousarray(
            a.reshape(R // 128, 128, C).transpose(1, 0, 2).reshape(128, -1)
        )

    in_maps = []
    for c in range(N_CORES):
        b, g = c // 2, c % 2
        gsl = slice(512 * g, 512 * (g + 1))
        in_maps.append(
            {
                "xq_t": np.ascontiguousarray(query[b].T.astype(bf)),
                "xk_t": np.ascontiguousarray(key[b].T.astype(bf)),
                "xv_t": np.ascontiguousarray(value[b].T.astype(bf)),
                "wq_t": ileave(Wq[gsl, :].T.astype(bf)),
                "wk_t": ileave(Wk[gsl, :].T.astype(bf)),
                "wv_t": ileave(Wv[gsl, :].T.astype(bf)),
                "wo_s": ileave(Wo[:, gsl].T.astype(bf)),
                "maskd": maskd,
                "sel8": sel8,
                "ones_col": ones_col,
            }
        )
    return in_maps


def _gather(results, bo, B):
    out = np.empty((B, S, D), np.float32)
    for b in range(B):
        out[b] = (
            results[2 * b]["out_p"]
            + results[2 * b + 1]["out_p"]
            + np.asarray(bo)[None, :]
        )
    return out


def _reference_fallback(query, key, value, mask, Wq, Wk, Wv, Wo, bo):
    B = query.shape[0]
    H = 16
    dk = D // H
    q = np.asarray(query, np.float32)
    k = np.asarray(key, np.float32)
    v = np.asarray(value, np.float32)

    def proj(x, W):
        return (x @ W.T).reshape(B, S, H, dk).transpose(0, 2, 1, 3)

    Q, K, V = proj(q, Wq), proj(k, Wk), proj(v, Wv)
    sc = np.einsum("bhqd,bhkd->bhqk", Q, K) / np.sqrt(np.float32(dk))
    sc = np.where(np.asarray(mask) == 0, np.float32(-1e9), sc)
    sc = sc - sc.max(axis=-1, keepdims=True)
    a = np.exp(sc)
    a = a / a.sum(axis=-1, keepdims=True)
    o = np.einsum("bhqk,bhkd->bhqd", a, V).transpose(0, 2, 1, 3).reshape(B, S, D)
    return (o @ np.asarray(Wo).T + np.asarray(bo)).astype(np.float32)


def kernel(query, key, value, mask, Wq, Wk, Wv, Wo, bo):
    query = np.asarray(query, np.float32)
    key = np.asarray(key, np.float32)
    value = np.asarray(value, np.float32)
    Wq, Wk, Wv, Wo = (np.asarray(w, np.float32) for w in (Wq, Wk, Wv, Wo))
    in_maps = _host_prep(query, key, value, mask, Wq, Wk, Wv, Wo)
    if in_maps is None:  # non-causal mask: host fallback
        return _reference_fallback(query, key, value, mask, Wq, Wk, Wv, Wo, bo)
    nc = _get(1)
    res = run_bass_kernel_spmd(nc, in_maps, list(range(N_CORES)))
    return _gather(res.results, bo, query.shape[0])


def run_spmd(in_maps, repeat=1):
    """For test.py: run prebuilt kernel, return BassKernelResults."""
    nc = _get(repeat)
    return run_bass_kernel_spmd(nc, in_maps, list(range(N_CORES)))


def host_prep(*args, **kw):
    return _host_prep(*args, **kw)


def gather(results, bo, B=4):
    return _gather(results, bo, B)


# revision 24
# speedup vs baseline: 1.0048x; 1.0048x over previous
"""Multi-head causal attention on 8 Trainium2 NeuronCores (Bass/Tile).

Problem: B=4, S=1024, D=1024, H=16 heads (dk=64), causal mask, fp32 I/O.

Sharding: 8 cores = 4 batches x 2 head-groups (8 heads each).
  Wq/Wk/Wv sharded column-wise by head (tensor parallel), Wo row-wise;
  the Wo all-reduce is a host-side pairwise sum (2 cores per batch).

Per-core kernel (bf16 matmul operands, fp32 PSUM accumulate, ~4.7e-3
absmax-relative vs the fp32 reference):
  phase P: Q^T (zero-padded per head: slot h holds Q_h^T on its 64
           partitions, zeros elsewhere, so score matmuls contract K=128
           at full rate against the packed K^T without mixing heads),
           K^T packed [128, 4, S], V -> v_sb [128, ki, head, 65] with a
           65th ones column per head (softmax denominator trick).
           PSUM->SBUF copies run on the otherwise-idle ACT engine.
  phase A: per head-chunk hc and q-half qj: scores^T [k=128, q<=512]
           (causally width-trimmed), exp on ACT (no max subtraction:
           |scores/8| < ~6), 0/1 mask multiply only on the diagonal
           128-block, attnV accumulated over k-chunks with lhsT =
           V_ext [k, 65]; row 64 = denominator. Denominator rows are
           copied on ACT and DMA-gathered into den8 (keeps the DVE FIFO
           and PE stream free of per-pair round-trips).
  phase O: two batched reciprocals, selector-matmul broadcast of 1/den
           over partition halves, in-place normalize of headout^T,
           output projection accumulating over d-chunks, DMA out.
"""

from contextlib import ExitStack

import ml_dtypes
import numpy as np

import concourse.bacc as bacc
import concourse.tile as tile
from concourse import mybir
from concourse.bass_utils import run_bass_kernel_spmd

A_SUB = {"mask", "av", "extract", "houtcp", "recip"}  # phase-A bisection knob

F32R = mybir.dt.float32r
F32 = mybir.dt.float32
BF16 = mybir.dt.bfloat16
EXP = mybir.ActivationFunctionType.Exp

S = 1024  # sequence length
D = 1024  # model dim
DK = 64  # head dim
HPC = 8  # heads per core
N_CORES = 8
SCALE = 1.0 / np.sqrt(DK)  # folded into the exp activation


def _emit(nc, tc, t, rep, phases=("P", "A", "O")):
    """Emit one full forward pass. `t` = dict of dram tensors."""
    ctx = ExitStack()
    with ctx:
        # ---- long-lived SBUF (per repeat; pools free at phase end) ----
        main = ctx.enter_context(tc.tile_pool(name=f"main{rep}", bufs=1))
        xpool = ctx.enter_context(tc.tile_pool(name=f"xin{rep}", bufs=2))
        wpool = ctx.enter_context(tc.tile_pool(name=f"win{rep}", bufs=2))

        # Q^T zero-padded per head: slot h holds Q_h^T on its 64 partitions,
        # zeros on the other 64 -> score matmuls contract K=128 (full rate)
        # against the packed kt_sb without mixing heads.
        qtz = main.tile([128, 8, S], BF16)
        kt_sb = main.tile([128, 4, S], BF16)
        v_sb = main.tile([128, 8, 8, 65], BF16)  # s-part: (ki, head, d+1)
        hout_sb = main.tile([128, 4, S], BF16)  # headout^T (unnormalized)
        maskd = main.tile([128, 128], BF16)  # diagonal-block 0/1 mask
        sel8 = main.tile([8, 512], F32R)
        den8 = main.tile([8, S], F32)
        rec8 = main.tile([8, S], F32R)
        wo_sb = main.tile([128, 4, S], BF16)

        nc.sync.dma_start(out=sel8, in_=t["sel8"][:, :])
        nc.sync.dma_start(out=maskd, in_=t["maskd"][:, :])
        nc.sync.dma_start(
            out=v_sb.rearrange("p a b c -> p (a b) c")[:, :, 64:65],
            in_=t["ones_col"][:, :, None],
        )
        nc.vector.memset(qtz.rearrange("p a b -> p (a b)"), 0.0)

        # ================= phase P: projections =================
        if "P" in phases:
         with (
            tc.tile_pool(name=f"pps{rep}", bufs=2, space="PSUM") as ppool,
        ):
            for which, xname, wname in (
                ("q", "xq_t", "wq_t"),
                ("k", "xk_t", "wk_t"),
            ):
                x_sb = xpool.tile([128, 8, S], BF16, tag="x")
                w_sb = wpool.tile([128, 8, 512], BF16, tag="w")
                xdr = t[xname].rearrange("(n p) s -> p n s", p=128)
                nc.sync.dma_start(
                    out=w_sb.rearrange("p a b -> p (a b)"), in_=t[wname][:, :]
                )
                for half in range(2):  # column halves: s-half chains start early
                    nc.sync.dma_start(
                        out=x_sb[:, :, 512 * half : 512 * (half + 1)],
                        in_=xdr[:, :, 512 * half : 512 * (half + 1)],
                    )
                if True:
                    for sj in range(2):
                        for dtile in range(4):
                            ps = ppool.tile([128, 512], F32, tag="ps")
                            for c in range(8):
                                nc.tensor.matmul(
                                    ps,
                                    w_sb[:, c, 128 * dtile : 128 * (dtile + 1)],
                                    x_sb[:, c, 512 * sj : 512 * (sj + 1)],
                                    start=(c == 0),
                                    stop=(c == 7),
                                )
                            sjs = slice(512 * sj, 512 * (sj + 1))
                            if which == "q":
                                nc.scalar.copy(
                                    qtz[0:64, 2 * dtile, sjs], ps[0:64, :]
                                )
                                nc.scalar.copy(
                                    qtz[64:128, 2 * dtile + 1, sjs], ps[64:128, :]
                                )
                            else:
                                nc.scalar.copy(kt_sb[:, dtile, sjs], ps)

        # ========= phase A + O fused: per q-half, attention then outproj ====
        if "A" in phases:
         with (
            tc.tile_pool(name=f"scps{rep}", bufs=2, space="PSUM") as scpool,
            tc.tile_pool(name=f"avps{rep}", bufs=2, space="PSUM") as avpool,
            tc.tile_pool(name=f"ops{rep}", bufs=2, space="PSUM") as opool,
            tc.tile_pool(name=f"epool{rep}", bufs=12) as epool,
            tc.tile_pool(name=f"xtr{rep}", bufs=6) as xtr,
            tc.tile_pool(name=f"osb{rep}", bufs=3) as osb,
        ):
            if "O" in phases:  # prefetch Wo during attention
                nc.sync.dma_start(
                    out=wo_sb.rearrange("p a b -> p (a b)"), in_=t["wo_s"][:, :]
                )
            # V projection shares the score pool so Q.K scores/exp overlap it
            xv_sb = xpool.tile([128, 8, S], BF16, tag="x")
            wv_sb = wpool.tile([128, 8, 512], BF16, tag="w")
            xvdr = t["xv_t"].rearrange("(n p) s -> p n s", p=128)
            nc.sync.dma_start(
                out=wv_sb.rearrange("p a b -> p (a b)"), in_=t["wv_t"][:, :]
            )
            for half in range(2):
                nc.sync.dma_start(
                    out=xv_sb[:, :, 512 * half : 512 * (half + 1)],
                    in_=xvdr[:, :, 512 * half : 512 * (half + 1)],
                )
            for tpair in range(4):
                ps2 = scpool.tile([128, 2, 512], F32, tag="sc")
                for sub in range(2):
                    stile = 2 * tpair + sub
                    for c in range(8):
                        nc.tensor.matmul(
                            ps2[:, sub, :],
                            xv_sb[:, c, 128 * stile : 128 * (stile + 1)],
                            wv_sb[:, c, :],
                            start=(c == 0),
                            stop=(c == 7),
                        )
                    nc.scalar.copy(
                        v_sb[:, stile, :, 0:64],
                        ps2[:, sub, :].rearrange("p (h c) -> p h c", c=64),
                    )
            for qj in range(2):
                qsl = slice(512 * qj, 512 * (qj + 1))
                for hc in range(4):
                    kmax = 4 if qj == 0 else 8
                    o_e = avpool.tile([128, 512], F32, tag="av")
                    o_o = avpool.tile([128, 512], F32, tag="av")
                    for ki in range(kmax):
                        # causal: columns [0, b) of this 512-block are fully
                        # masked for this k-chunk; diagonal block at [b, b+128)
                        b = 128 * max(0, ki - 4 * qj)
                        kis = slice(128 * ki, 128 * (ki + 1))
                        sc = scpool.tile([128, 2, 512], F32, tag="sc")
                        nc.tensor.matmul(
                            sc[:, 0, b:512],
                            kt_sb[:, hc, kis],
                            qtz[:, 2 * hc, 512 * qj + b : 512 * (qj + 1)],
                            start=True,
                            stop=True,
                        )
                        nc.tensor.matmul(
                            sc[:, 1, b:512],
                            kt_sb[:, hc, kis],
                            qtz[:, 2 * hc + 1, 512 * qj + b : 512 * (qj + 1)],
                            start=True,
                            stop=True,
                        )
                        ee = epool.tile([128, 2, 512], BF16, tag="e")
                        nc.scalar.activation(
                            ee[:, :, b:512],
                            sc[:, :, b:512],
                            EXP,
                            scale=float(SCALE),
                        )
                        if ki - 4 * qj >= 0:  # diagonal block: 0/1 mask
                            nc.vector.tensor_mul(
                                ee[:, 0, b : b + 128], ee[:, 0, b : b + 128], maskd
                            )
                            nc.vector.tensor_mul(
                                ee[:, 1, b : b + 128], ee[:, 1, b : b + 128], maskd
                            )
                        nc.tensor.matmul(
                            o_e[0:65, b:512],
                            v_sb[:, ki, 2 * hc, :],
                            ee[:, 0, b:512],
                            start=(ki == 0),
                            stop=(ki == kmax - 1),
                        )
                        nc.tensor.matmul(
                            o_o[0:65, b:512],
                            v_sb[:, ki, 2 * hc + 1, :],
                            ee[:, 1, b:512],
                            start=(ki == 0),
                            stop=(ki == kmax - 1),
                        )
                    # extract headout^T + denominator rows
                    nc.vector.tensor_copy(hout_sb[0:64, hc, qsl], o_e[0:64, :])
                    otmp = xtr.tile([64, 512], BF16, tag="otmp")
                    nc.vector.tensor_copy(otmp, o_o[0:64, :])
                    nc.sync.dma_start(out=hout_sb[64:128, hc, qsl], in_=otmp)
                    de_t = xtr.tile([1, 512], F32, tag="de")
                    do_t = xtr.tile([1, 512], F32, tag="do")
                    nc.vector.tensor_copy(de_t, o_e[64:65, :])
                    nc.vector.tensor_copy(do_t, o_o[64:65, :])
                    nc.sync.dma_start(out=den8[2 * hc : 2 * hc + 1, qsl], in_=de_t)
                    nc.sync.dma_start(
                        out=den8[2 * hc + 1 : 2 * hc + 2, qsl], in_=do_t
                    )
            for qj in range(2) if "O" in phases else []:
                qsl = slice(512 * qj, 512 * (qj + 1))
                # ---- normalize + output projection for this q-half
                with nc.allow_low_precision(reason="softmax reciprocal"):
                    nc.vector.reciprocal(rec8[:, qsl], den8[:, qsl])
                for hc in range(4):
                    bp = opool.tile([128, 512], F32, tag="op")
                    nc.tensor.matmul(
                        bp,
                        sel8[:, 128 * hc : 128 * (hc + 1)],
                        rec8[:, qsl],
                        start=True,
                        stop=True,
                    )
                    nc.vector.tensor_mul(
                        hout_sb[:, hc, qsl], hout_sb[:, hc, qsl], bp
                    )
                for stile in range(4 * qj, 4 * qj + 4):
                    out_sb = osb.tile([128, S], F32, tag="out")
                    for ej in range(2):
                        fp = opool.tile([128, 512], F32, tag="op")
                        for hc in range(4):
                            nc.tensor.matmul(
                                fp,
                                hout_sb[:, hc, 128 * stile : 128 * (stile + 1)],
                                wo_sb[:, hc, 512 * ej : 512 * (ej + 1)],
                                start=(hc == 0),
                                stop=(hc == 3),
                            )
                        esl = slice(512 * ej, 512 * (ej + 1))
                        if ej == 0:
                            nc.vector.tensor_copy(out_sb[:, esl], fp)
                        else:
                            nc.scalar.copy(out_sb[:, esl], fp)
                    nc.sync.dma_start(
                        out=t["out_p"][128 * stile : 128 * (stile + 1), :],
                        in_=out_sb,
                    )


def _build_phases(phases, repeat=1):
    return _build(repeat, phases=phases)


def _build(repeat=1, phases=("P", "A", "O")):
    nc = bacc.Bacc()
    t = {}
    for name in ("xq_t", "xk_t", "xv_t"):
        t[name] = nc.dram_tensor(name, [D, S], BF16, kind="ExternalInput")
    for name in ("wq_t", "wk_t", "wv_t"):
        t[name] = nc.dram_tensor(name, [128, 8 * 512], BF16, kind="ExternalInput")
    t["wo_s"] = nc.dram_tensor("wo_s", [128, 4 * D], BF16, kind="ExternalInput")
    t["maskd"] = nc.dram_tensor("maskd", [128, 128], BF16, kind="ExternalInput")
    t["sel8"] = nc.dram_tensor("sel8", [8, 512], F32R, kind="ExternalInput")
    t["ones_col"] = nc.dram_tensor("ones_col", [128, 64], BF16, kind="ExternalInput")
    t["out_p"] = nc.dram_tensor("out_p", [S, D], F32, kind="ExternalOutput")

    with tile.TileContext(nc) as tc:
        if repeat == 1:
            _emit(nc, tc, t, 0, phases)
        else:
            with tc.For_i(0, repeat, 1):
                _emit(nc, tc, t, 0, phases)
    nc.compile()
    return nc


_CACHE = {}


def _get(repeat=1):
    if repeat not in _CACHE:
        _CACHE[repeat] = _build(repeat)
    return _CACHE[repeat]


def _host_prep(query, key, value, mask, Wq, Wk, Wv, Wo):
    """Build the per-core in_maps. Returns None if mask isn't causal tril."""
    m = np.asarray(mask)[0, 0]
    if not np.array_equal(m, np.tril(np.ones((S, S), m.dtype))):
        return None

    bf = ml_dtypes.bfloat16

    # diagonal-block mask (same for every diagonal tile under causal tril)
    maskd = m[0:128, 0:128].T.astype(bf)

    sel8 = np.zeros((8, 512), np.float32)
    for hc in range(4):
        sel8[2 * hc, 128 * hc : 128 * hc + 64] = 1.0
        sel8[2 * hc + 1, 128 * hc + 64 : 128 * hc + 128] = 1.0
    ones_col = np.ones((128, 64), bf)

    def ileave(a):  # [R, C] -> [128, (R//128)*C]: chunk-c data contiguous per p
        R, C = a.shape
        return np.ascontiguousarray(
            a.reshape(R // 128, 128, C).transpose(1, 0, 2).reshape(128, -1)
        )

    in_maps = []
    for c in range(N_CORES):
        b, g = c // 2, c % 2
        gsl = slice(512 * g, 512 * (g + 1))
        in_maps.append(
            {
                "xq_t": np.ascontiguousarray(query[b].T.astype(bf)),
                "xk_t": np.ascontiguousarray(key[b].T.astype(bf)),
                "xv_t": np.ascontiguousarray(value[b].T.astype(bf)),
                "wq_t": ileave(Wq[gsl, :].T.astype(bf)),
                "wk_t": ileave(Wk[gsl, :].T.astype(bf)),
                "wv_t": ileave(Wv[gsl, :].T.astype(bf)),
                "wo_s": ileave(Wo[:, gsl].T.astype(bf)),
                "maskd": maskd,
                "sel8": sel8,
                "ones_col": ones_col,
            }
        )
    return in_maps


def _gather(results, bo, B):
    out = np.empty((B, S, D), np.float32)
    for b in range(B):
        out[b] = (
            results[2 * b]["out_p"]
            + results[2 * b + 1]["out_p"]
            + np.asarray(bo)[None, :]
        )
    return out


def _reference_fallback(query, key, value, mask, Wq, Wk, Wv, Wo, bo):
    B = query.shape[0]
    H = 16
    dk = D // H
    q = np.asarray(query, np.float32)
    k = np.asarray(key, np.float32)
    v = np.asarray(value, np.float32)

    def proj(x, W):
        return (x @ W.T).reshape(B, S, H, dk).transpose(0, 2, 1, 3)

    Q, K, V = proj(q, Wq), proj(k, Wk), proj(v, Wv)
    sc = np.einsum("bhqd,bhkd->bhqk", Q, K) / np.sqrt(np.float32(dk))
    sc = np.where(np.asarray(mask) == 0, np.float32(-1e9), sc)
    sc = sc - sc.max(axis=-1, keepdims=True)
    a = np.exp(sc)
    a = a / a.sum(axis=-1, keepdims=True)
    o = np.einsum("bhqk,bhkd->bhqd", a, V).transpose(0, 2, 1, 3).reshape(B, S, D)
    return (o @ np.asarray(Wo).T + np.asarray(bo)).astype(np.float32)


def kernel(query, key, value, mask, Wq, Wk, Wv, Wo, bo):
    query = np.asarray(query, np.float32)
    key = np.asarray(key, np.float32)
    value = np.asarray(value, np.float32)
    Wq, Wk, Wv, Wo = (np.asarray(w, np.float32) for w in (Wq, Wk, Wv, Wo))
    in_maps = _host_prep(query, key, value, mask, Wq, Wk, Wv, Wo)
    if in_maps is None:  # non-causal mask: host fallback
        return _reference_fallback(query, key, value, mask, Wq, Wk, Wv, Wo, bo)
    nc = _get(1)
    res = run_bass_kernel_spmd(nc, in_maps, list(range(N_CORES)))
    return _gather(res.results, bo, query.shape[0])


def run_spmd(in_maps, repeat=1):
    """For test.py: run prebuilt kernel, return BassKernelResults."""
    nc = _get(repeat)
    return run_bass_kernel_spmd(nc, in_maps, list(range(N_CORES)))


def host_prep(*args, **kw):
    return _host_prep(*args, **kw)


def gather(results, bo, B=4):
    return _gather(results, bo, B)


# revision 25
# speedup vs baseline: 1.0166x; 1.0118x over previous
"""Multi-head causal attention on 8 Trainium2 NeuronCores (Bass/Tile).

Problem: B=4, S=1024, D=1024, H=16 heads (dk=64), causal mask, fp32 I/O.

Sharding: 8 cores = 4 batches x 2 head-groups (8 heads each).
  Wq/Wk/Wv sharded column-wise by head (tensor parallel), Wo row-wise;
  the Wo all-reduce is a host-side pairwise sum (2 cores per batch).

Per-core kernel (bf16 matmul operands, fp32 PSUM accumulate, ~4.7e-3
absmax-relative vs the fp32 reference):
  phase P: Q^T (zero-padded per head: slot h holds Q_h^T on its 64
           partitions, zeros elsewhere, so score matmuls contract K=128
           at full rate against the packed K^T without mixing heads),
           K^T packed [128, 4, S], V -> v_sb [128, ki, head, 65] with a
           65th ones column per head (softmax denominator trick).
           PSUM->SBUF copies run on the otherwise-idle ACT engine.
  phase A: per head-chunk hc and q-half qj: scores^T [k=128, q<=512]
           (causally width-trimmed), exp on ACT (no max subtraction:
           |scores/8| < ~6), 0/1 mask multiply only on the diagonal
           128-block, attnV accumulated over k-chunks with lhsT =
           V_ext [k, 65]; row 64 = denominator. Denominator rows are
           copied on ACT and DMA-gathered into den8 (keeps the DVE FIFO
           and PE stream free of per-pair round-trips).
  phase O: two batched reciprocals, selector-matmul broadcast of 1/den
           over partition halves, in-place normalize of headout^T,
           output projection accumulating over d-chunks, DMA out.
"""

from contextlib import ExitStack

import ml_dtypes
import numpy as np

import concourse.bacc as bacc
import concourse.tile as tile
from concourse import mybir
from concourse.bass_utils import run_bass_kernel_spmd

A_SUB = {"mask", "av", "extract", "houtcp", "recip"}  # phase-A bisection knob

F32R = mybir.dt.float32r
F32 = mybir.dt.float32
BF16 = mybir.dt.bfloat16
EXP = mybir.ActivationFunctionType.Exp

S = 1024  # sequence length
D = 1024  # model dim
DK = 64  # head dim
HPC = 8  # heads per core
N_CORES = 8
SCALE = 1.0 / np.sqrt(DK)  # folded into the exp activation


def _emit(nc, tc, t, rep, phases=("P", "A", "O")):
    """Emit one full forward pass. `t` = dict of dram tensors."""
    ctx = ExitStack()
    with ctx:
        # ---- long-lived SBUF (per repeat; pools free at phase end) ----
        main = ctx.enter_context(tc.tile_pool(name=f"main{rep}", bufs=1))
        xpool = ctx.enter_context(tc.tile_pool(name=f"xin{rep}", bufs=2))
        wpool = ctx.enter_context(tc.tile_pool(name=f"win{rep}", bufs=2))

        # Q^T zero-padded per head: slot h holds Q_h^T on its 64 partitions,
        # zeros on the other 64 -> score matmuls contract K=128 (full rate)
        # against the packed kt_sb without mixing heads.
        qtz = main.tile([128, 8, S], BF16)
        kt_sb = main.tile([128, 4, S], BF16)
        v_sb = main.tile([128, 8, 8, 65], BF16)  # s-part: (ki, head, d+1)
        hout_sb = main.tile([128, 4, S], BF16)  # headout^T (unnormalized)
        maskd = main.tile([128, 128], BF16)  # diagonal-block 0/1 mask
        sel8 = main.tile([8, 512], F32R)
        den8 = main.tile([8, S], F32)
        rec8 = main.tile([8, S], F32R)
        wo_sb = main.tile([128, 4, S], BF16)

        nc.sync.dma_start(out=sel8, in_=t["sel8"][:, :])
        nc.sync.dma_start(out=maskd, in_=t["maskd"][:, :])
        nc.sync.dma_start(
            out=v_sb.rearrange("p a b c -> p (a b) c")[:, :, 64:65],
            in_=t["ones_col"][:, :, None],
        )
        nc.vector.memset(qtz.rearrange("p a b -> p (a b)"), 0.0)

        # ================= phase P: projections =================
        if "P" in phases:
         with (
            tc.tile_pool(name=f"pps{rep}", bufs=2, space="PSUM") as ppool,
        ):
            for which, xname, wname in (
                ("q", "xq_t", "wq_t"),
                ("k", "xk_t", "wk_t"),
            ):
                x_sb = xpool.tile([128, 8, S], BF16, tag="x")
                w_sb = wpool.tile([128, 8, 512], BF16, tag="w")
                xdr = t[xname].rearrange("(n p) s -> p n s", p=128)
                nc.sync.dma_start(
                    out=w_sb.rearrange("p a b -> p (a b)"), in_=t[wname][:, :]
                )
                for half in range(2):  # column halves: s-half chains start early
                    nc.sync.dma_start(
                        out=x_sb[:, :, 512 * half : 512 * (half + 1)],
                        in_=xdr[:, :, 512 * half : 512 * (half + 1)],
                    )
                if True:
                    for sj in range(2):
                        for dtile in range(4):
                            ps = ppool.tile([128, 512], F32, tag="ps")
                            for c in range(8):
                                nc.tensor.matmul(
                                    ps,
                                    w_sb[:, c, 128 * dtile : 128 * (dtile + 1)],
                                    x_sb[:, c, 512 * sj : 512 * (sj + 1)],
                                    start=(c == 0),
                                    stop=(c == 7),
                                )
                            sjs = slice(512 * sj, 512 * (sj + 1))
                            if which == "q":
                                nc.scalar.copy(
                                    qtz[0:64, 2 * dtile, sjs], ps[0:64, :]
                                )
                                nc.scalar.copy(
                                    qtz[64:128, 2 * dtile + 1, sjs], ps[64:128, :]
                                )
                            else:
                                nc.scalar.copy(kt_sb[:, dtile, sjs], ps)

        # ========= phase A + O fused: per q-half, attention then outproj ====
        if "A" in phases:
         with (
            tc.tile_pool(name=f"scps{rep}", bufs=2, space="PSUM") as scpool,
            tc.tile_pool(name=f"avps{rep}", bufs=2, space="PSUM") as avpool,
            tc.tile_pool(name=f"ops{rep}", bufs=2, space="PSUM") as opool,
            tc.tile_pool(name=f"epool{rep}", bufs=12) as epool,
            tc.tile_pool(name=f"xtr{rep}", bufs=6) as xtr,
            tc.tile_pool(name=f"osb{rep}", bufs=3) as osb,
        ):
            if "O" in phases:  # prefetch Wo during attention
                nc.sync.dma_start(
                    out=wo_sb.rearrange("p a b -> p (a b)"), in_=t["wo_s"][:, :]
                )
            # V projection shares the score pool so Q.K scores/exp overlap it
            xv_sb = xpool.tile([128, 8, S], BF16, tag="x")
            wv_sb = wpool.tile([128, 8, 512], BF16, tag="w")
            xvdr = t["xv_t"].rearrange("(n p) s -> p n s", p=128)
            nc.sync.dma_start(
                out=wv_sb.rearrange("p a b -> p (a b)"), in_=t["wv_t"][:, :]
            )
            for half in range(2):
                nc.sync.dma_start(
                    out=xv_sb[:, :, 512 * half : 512 * (half + 1)],
                    in_=xvdr[:, :, 512 * half : 512 * (half + 1)],
                )
            for tpair in range(4):
                ps2 = scpool.tile([128, 2, 512], F32, tag="sc")
                for sub in range(2):
                    stile = 2 * tpair + sub
                    for c in range(8):
                        nc.tensor.matmul(
                            ps2[:, sub, :],
                            xv_sb[:, c, 128 * stile : 128 * (stile + 1)],
                            wv_sb[:, c, :],
                            start=(c == 0),
                            stop=(c == 7),
                        )
                    nc.scalar.copy(
                        v_sb[:, stile, :, 0:64],
                        ps2[:, sub, :].rearrange("p (h c) -> p h c", c=64),
                    )
            for qj in range(2):
                qsl = slice(512 * qj, 512 * (qj + 1))
                for hc in range(4):
                    kmax = 4 if qj == 0 else 8
                    o_e = avpool.tile([128, 512], F32, tag="av")
                    o_o = avpool.tile([128, 512], F32, tag="av")
                    def emit_score(ki):
                        b = 128 * max(0, ki - 4 * qj)
                        kis = slice(128 * ki, 128 * (ki + 1))
                        sc = scpool.tile([128, 2, 512], F32, tag="sc")
                        nc.tensor.matmul(
                            sc[:, 0, b:512],
                            kt_sb[:, hc, kis],
                            qtz[:, 2 * hc, 512 * qj + b : 512 * (qj + 1)],
                            start=True,
                            stop=True,
                        )
                        nc.tensor.matmul(
                            sc[:, 1, b:512],
                            kt_sb[:, hc, kis],
                            qtz[:, 2 * hc + 1, 512 * qj + b : 512 * (qj + 1)],
                            start=True,
                            stop=True,
                        )
                        return sc

                    scs = {0: emit_score(0)}
                    for ki in range(kmax):
                        # software pipeline: next scores ahead of this attnV
                        if ki + 1 < kmax:
                            scs[ki + 1] = emit_score(ki + 1)
                        b = 128 * max(0, ki - 4 * qj)
                        sc = scs.pop(ki)
                        ee = epool.tile([128, 2, 512], BF16, tag="e")
                        nc.scalar.activation(
                            ee[:, :, b:512],
                            sc[:, :, b:512],
                            EXP,
                            scale=float(SCALE),
                        )
                        if ki - 4 * qj >= 0:  # diagonal block: 0/1 mask
                            nc.vector.tensor_mul(
                                ee[:, 0, b : b + 128], ee[:, 0, b : b + 128], maskd
                            )
                            nc.vector.tensor_mul(
                                ee[:, 1, b : b + 128], ee[:, 1, b : b + 128], maskd
                            )
                        nc.tensor.matmul(
                            o_e[0:65, b:512],
                            v_sb[:, ki, 2 * hc, :],
                            ee[:, 0, b:512],
                            start=(ki == 0),
                            stop=(ki == kmax - 1),
                        )
                        nc.tensor.matmul(
                            o_o[0:65, b:512],
                            v_sb[:, ki, 2 * hc + 1, :],
                            ee[:, 1, b:512],
                            start=(ki == 0),
                            stop=(ki == kmax - 1),
                        )
                    # extract headout^T + denominator rows
                    nc.vector.tensor_copy(hout_sb[0:64, hc, qsl], o_e[0:64, :])
                    otmp = xtr.tile([64, 512], BF16, tag="otmp")
                    nc.vector.tensor_copy(otmp, o_o[0:64, :])
                    nc.sync.dma_start(out=hout_sb[64:128, hc, qsl], in_=otmp)
                    de_t = xtr.tile([1, 512], F32, tag="de")
                    do_t = xtr.tile([1, 512], F32, tag="do")
                    nc.vector.tensor_copy(de_t, o_e[64:65, :])
                    nc.vector.tensor_copy(do_t, o_o[64:65, :])
                    nc.sync.dma_start(out=den8[2 * hc : 2 * hc + 1, qsl], in_=de_t)
                    nc.sync.dma_start(
                        out=den8[2 * hc + 1 : 2 * hc + 2, qsl], in_=do_t
                    )
            for qj in range(2) if "O" in phases else []:
                qsl = slice(512 * qj, 512 * (qj + 1))
                # ---- normalize + output projection for this q-half
                with nc.allow_low_precision(reason="softmax reciprocal"):
                    nc.vector.reciprocal(rec8[:, qsl], den8[:, qsl])
                for hc in range(4):
                    bp = opool.tile([128, 512], F32, tag="op")
                    nc.tensor.matmul(
                        bp,
                        sel8[:, 128 * hc : 128 * (hc + 1)],
                        rec8[:, qsl],
                        start=True,
                        stop=True,
                    )
                    nc.vector.tensor_mul(
                        hout_sb[:, hc, qsl], hout_sb[:, hc, qsl], bp
                    )
                for stile in range(4 * qj, 4 * qj + 4):
                    out_sb = osb.tile([128, S], F32, tag="out")
                    for ej in range(2):
                        fp = opool.tile([128, 512], F32, tag="op")
                        for hc in range(4):
                            nc.tensor.matmul(
                                fp,
                                hout_sb[:, hc, 128 * stile : 128 * (stile + 1)],
                                wo_sb[:, hc, 512 * ej : 512 * (ej + 1)],
                                start=(hc == 0),
                                stop=(hc == 3),
                            )
                        esl = slice(512 * ej, 512 * (ej + 1))
                        if ej == 0:
                            nc.vector.tensor_copy(out_sb[:, esl], fp)
                        else:
                            nc.scalar.copy(out_sb[:, esl], fp)
                    nc.sync.dma_start(
                        out=t["out_p"][128 * stile : 128 * (stile + 1), :],
                        in_=out_sb,
                    )


def _build_phases(phases, repeat=1):
    return _build(repeat, phases=phases)


def _build(repeat=1, phases=("P", "A", "O")):
    nc = bacc.Bacc()
    t = {}
    for name in ("xq_t", "xk_t", "xv_t"):
        t[name] = nc.dram_tensor(name, [D, S], BF16, kind="ExternalInput")
    for name in ("wq_t", "wk_t", "wv_t"):
        t[name] = nc.dram_tensor(name, [128, 8 * 512], BF16, kind="ExternalInput")
    t["wo_s"] = nc.dram_tensor("wo_s", [128, 4 * D], BF16, kind="ExternalInput")
    t["maskd"] = nc.dram_tensor("maskd", [128, 128], BF16, kind="ExternalInput")
    t["sel8"] = nc.dram_tensor("sel8", [8, 512], F32R, kind="ExternalInput")
    t["ones_col"] = nc.dram_tensor("ones_col", [128, 64], BF16, kind="ExternalInput")
    t["out_p"] = nc.dram_tensor("out_p", [S, D], F32, kind="ExternalOutput")

    with tile.TileContext(nc) as tc:
        if repeat == 1:
            _emit(nc, tc, t, 0, phases)
        else:
            with tc.For_i(0, repeat, 1):
                _emit(nc, tc, t, 0, phases)
    nc.compile()
    return nc


_CACHE = {}


def _get(repeat=1):
    if repeat not in _CACHE:
        _CACHE[repeat] = _build(repeat)
    return _CACHE[repeat]


def _host_prep(query, key, value, mask, Wq, Wk, Wv, Wo):
    """Build the per-core in_maps. Returns None if mask isn't causal tril."""
    m = np.asarray(mask)[0, 0]
    if not np.array_equal(m, np.tril(np.ones((S, S), m.dtype))):
        return None

    bf = ml_dtypes.bfloat16

    # diagonal-block mask (same for every diagonal tile under causal tril)
    maskd = m[0:128, 0:128].T.astype(bf)

    sel8 = np.zeros((8, 512), np.float32)
    for hc in range(4):
        sel8[2 * hc, 128 * hc : 128 * hc + 64] = 1.0
        sel8[2 * hc + 1, 128 * hc + 64 : 128 * hc + 128] = 1.0
    ones_col = np.ones((128, 64), bf)

    def ileave(a):  # [R, C] -> [128, (R//128)*C]: chunk-c data contiguous per p
        R, C = a.shape
        return np.ascontiguousarray(
            a.reshape(R // 128, 128, C).transpose(1, 0, 2).reshape(128, -1)
        )

    in_maps = []
    for c in range(N_CORES):
        b, g = c // 2, c % 2
        gsl = slice(512 * g, 512 * (g + 1))
        in_maps.append(
            {
                "xq_t": np.ascontiguousarray(query[b].T.astype(bf)),
                "xk_t": np.ascontiguousarray(key[b].T.astype(bf)),
                "xv_t": np.ascontiguousarray(value[b].T.astype(bf)),
                "wq_t": ileave(Wq[gsl, :].T.astype(bf)),
                "wk_t": ileave(Wk[gsl, :].T.astype(bf)),
                "wv_t": ileave(Wv[gsl, :].T.astype(bf)),
                "wo_s": ileave(Wo[:, gsl].T.astype(bf)),
                "maskd": maskd,
                "sel8": sel8,
                "ones_col": ones_col,
            }
        )
    return in_maps


def _gather(results, bo, B):
    out = np.empty((B, S, D), np.float32)
    for b in range(B):
        out[b] = (
            results[2 * b]["out_p"]
            + results[2 * b + 1]["out_p"]
            + np.asarray(bo)[None, :]
        )
    return out


def _reference_fallback(query, key, value, mask, Wq, Wk, Wv, Wo, bo):
    B = query.shape[0]
    H = 16
    dk = D // H
    q = np.asarray(query, np.float32)
    k = np.asarray(key, np.float32)
    v = np.asarray(value, np.float32)

    def proj(x, W):
        return (x @ W.T).reshape(B, S, H, dk).transpose(0, 2, 1, 3)

    Q, K, V = proj(q, Wq), proj(k, Wk), proj(v, Wv)
    sc = np.einsum("bhqd,bhkd->bhqk", Q, K) / np.sqrt(np.float32(dk))
    sc = np.where(np.asarray(mask) == 0, np.float32(-1e9), sc)
    sc = sc - sc.max(axis=-1, keepdims=True)
    a = np.exp(sc)
    a = a / a.sum(axis=-1, keepdims=True)
    o = np.einsum("bhqk,bhkd->bhqd", a, V).transpose(0, 2, 1, 3).reshape(B, S, D)
    return (o @ np.asarray(Wo).T + np.asarray(bo)).astype(np.float32)


def kernel(query, key, value, mask, Wq, Wk, Wv, Wo, bo):
    query = np.asarray(query, np.float32)
    key = np.asarray(key, np.float32)
    value = np.asarray(value, np.float32)
    Wq, Wk, Wv, Wo = (np.asarray(w, np.float32) for w in (Wq, Wk, Wv, Wo))
    in_maps = _host_prep(query, key, value, mask, Wq, Wk, Wv, Wo)
    if in_maps is None:  # non-causal mask: host fallback
        return _reference_fallback(query, key, value, mask, Wq, Wk, Wv, Wo, bo)
    nc = _get(1)
    res = run_bass_kernel_spmd(nc, in_maps, list(range(N_CORES)))
    return _gather(res.results, bo, query.shape[0])


def run_spmd(in_maps, repeat=1):
    """For test.py: run prebuilt kernel, return BassKernelResults."""
    nc = _get(repeat)
    return run_bass_kernel_spmd(nc, in_maps, list(range(N_CORES)))


def host_prep(*args, **kw):
    return _host_prep(*args, **kw)


def gather(results, bo, B=4):
    return _gather(results, bo, B)


# revision 27
# speedup vs baseline: 1.0363x; 1.0194x over previous
"""Multi-head causal attention on 8 Trainium2 NeuronCores (Bass/Tile).

Problem: B=4, S=1024, D=1024, H=16 heads (dk=64), causal mask, fp32 I/O.

Sharding: 8 cores = 4 batches x 2 head-groups (8 heads each).
  Wq/Wk/Wv sharded column-wise by head (tensor parallel), Wo row-wise;
  the Wo all-reduce is a host-side pairwise sum (2 cores per batch).

Per-core kernel (bf16 matmul operands, fp32 PSUM accumulate, ~4.7e-3
absmax-relative vs the fp32 reference):
  phase P: Q^T (zero-padded per head: slot h holds Q_h^T on its 64
           partitions, zeros elsewhere, so score matmuls contract K=128
           at full rate against the packed K^T without mixing heads),
           K^T packed [128, 4, S], V -> v_sb [128, ki, head, 65] with a
           65th ones column per head (softmax denominator trick).
           PSUM->SBUF copies run on the otherwise-idle ACT engine.
  phase A: per head-chunk hc and q-half qj: scores^T [k=128, q<=512]
           (causally width-trimmed), exp on ACT (no max subtraction:
           |scores/8| < ~6), 0/1 mask multiply only on the diagonal
           128-block, attnV accumulated over k-chunks with lhsT =
           V_ext [k, 65]; row 64 = denominator. Denominator rows are
           copied on ACT and DMA-gathered into den8 (keeps the DVE FIFO
           and PE stream free of per-pair round-trips).
  phase O: two batched reciprocals, selector-matmul broadcast of 1/den
           over partition halves, in-place normalize of headout^T,
           output projection accumulating over d-chunks, DMA out.
"""

from contextlib import ExitStack

import ml_dtypes
import numpy as np

import concourse.bacc as bacc
import concourse.tile as tile
from concourse import mybir
from concourse.bass_utils import run_bass_kernel_spmd

A_SUB = {"mask", "av", "extract", "houtcp", "recip"}  # phase-A bisection knob

F32R = mybir.dt.float32r
F32 = mybir.dt.float32
BF16 = mybir.dt.bfloat16
EXP = mybir.ActivationFunctionType.Exp

S = 1024  # sequence length
D = 1024  # model dim
DK = 64  # head dim
HPC = 8  # heads per core
N_CORES = 8
SCALE = 1.0 / np.sqrt(DK)  # folded into the exp activation


def _emit(nc, tc, t, rep, phases=("P", "A", "O")):
    """Emit one full forward pass. `t` = dict of dram tensors."""
    ctx = ExitStack()
    with ctx:
        # ---- long-lived SBUF (per repeat; pools free at phase end) ----
        main = ctx.enter_context(tc.tile_pool(name=f"main{rep}", bufs=1))
        xpool = ctx.enter_context(tc.tile_pool(name=f"xin{rep}", bufs=2))
        wpool = ctx.enter_context(tc.tile_pool(name=f"win{rep}", bufs=2))

        # Q^T zero-padded per head: slot h holds Q_h^T on its 64 partitions,
        # zeros on the other 64 -> score matmuls contract K=128 (full rate)
        # against the packed kt_sb without mixing heads.
        qtz = main.tile([128, 8, S], BF16)
        kt_sb = main.tile([128, 4, S], BF16)
        v_sb = main.tile([128, 8, 8, 65], BF16)  # s-part: (ki, head, d+1)
        hout_sb = main.tile([128, 4, S], BF16)  # headout^T (unnormalized)
        maskd = main.tile([128, 128], BF16)  # diagonal-block 0/1 mask
        sel8 = main.tile([8, 512], F32R)
        den8 = main.tile([8, S], F32)
        rec8 = main.tile([8, S], F32R)
        wo_sb = main.tile([128, 4, S], BF16)

        nc.sync.dma_start(out=sel8, in_=t["sel8"][:, :])
        nc.sync.dma_start(out=maskd, in_=t["maskd"][:, :])
        nc.sync.dma_start(
            out=v_sb.rearrange("p a b c -> p (a b) c")[:, :, 64:65],
            in_=t["ones_col"][:, :, None],
        )
        nc.vector.memset(qtz.rearrange("p a b -> p (a b)"), 0.0)

        # ================= phase P: projections =================
        if "P" in phases:
         with (
            tc.tile_pool(name=f"pps{rep}", bufs=2, space="PSUM") as ppool,
        ):
            for which, xname, wname in (
                ("q", "xq_t", "wq_t"),
                ("k", "xk_t", "wk_t"),
            ):
                x_sb = xpool.tile([128, 8, S], BF16, tag="x")
                w_sb = wpool.tile([128, 8, 512], BF16, tag="w")
                xdr = t[xname].rearrange("(n p) s -> p n s", p=128)
                nc.sync.dma_start(
                    out=w_sb.rearrange("p a b -> p (a b)"), in_=t[wname][:, :]
                )
                for half in range(2):  # column halves: s-half chains start early
                    nc.sync.dma_start(
                        out=x_sb[:, :, 512 * half : 512 * (half + 1)],
                        in_=xdr[:, :, 512 * half : 512 * (half + 1)],
                    )
                if True:
                    for sj in range(2):
                        for dtile in range(4):
                            ps = ppool.tile([128, 512], F32, tag="ps")
                            for c in range(8):
                                nc.tensor.matmul(
                                    ps,
                                    w_sb[:, c, 128 * dtile : 128 * (dtile + 1)],
                                    x_sb[:, c, 512 * sj : 512 * (sj + 1)],
                                    start=(c == 0),
                                    stop=(c == 7),
                                )
                            sjs = slice(512 * sj, 512 * (sj + 1))
                            if which == "q":
                                nc.scalar.copy(
                                    qtz[0:64, 2 * dtile, sjs], ps[0:64, :]
                                )
                                nc.scalar.copy(
                                    qtz[64:128, 2 * dtile + 1, sjs], ps[64:128, :]
                                )
                            else:
                                nc.scalar.copy(kt_sb[:, dtile, sjs], ps)

        # ========= phase A + O fused: per q-half, attention then outproj ====
        if "A" in phases:
         with (
            tc.tile_pool(name=f"scps{rep}", bufs=2, space="PSUM") as scpool,
            tc.tile_pool(name=f"avps{rep}", bufs=2, space="PSUM") as avpool,
            tc.tile_pool(name=f"ops{rep}", bufs=2, space="PSUM") as opool,
            tc.tile_pool(name=f"epool{rep}", bufs=12) as epool,
            tc.tile_pool(name=f"xtr{rep}", bufs=6) as xtr,
            tc.tile_pool(name=f"osb{rep}", bufs=3) as osb,
        ):
            if "O" in phases:  # prefetch Wo during attention
                nc.sync.dma_start(
                    out=wo_sb.rearrange("p a b -> p (a b)"), in_=t["wo_s"][:, :]
                )
            # V projection shares the score pool so Q.K scores/exp overlap it
            xv_sb = xpool.tile([128, 8, S], BF16, tag="x")
            wv_sb = wpool.tile([128, 8, 512], BF16, tag="w")
            xvdr = t["xv_t"].rearrange("(n p) s -> p n s", p=128)
            nc.sync.dma_start(
                out=wv_sb.rearrange("p a b -> p (a b)"), in_=t["wv_t"][:, :]
            )
            for half in range(2):
                nc.sync.dma_start(
                    out=xv_sb[:, :, 512 * half : 512 * (half + 1)],
                    in_=xvdr[:, :, 512 * half : 512 * (half + 1)],
                )
            for tpair in range(4):
                ps2 = scpool.tile([128, 2, 512], F32, tag="sc")
                for sub in range(2):
                    stile = 2 * tpair + sub
                    for c in range(8):
                        nc.tensor.matmul(
                            ps2[:, sub, :],
                            xv_sb[:, c, 128 * stile : 128 * (stile + 1)],
                            wv_sb[:, c, :],
                            start=(c == 0),
                            stop=(c == 7),
                        )
                    nc.scalar.copy(
                        v_sb[:, stile, :, 0:64],
                        ps2[:, sub, :].rearrange("p (h c) -> p h c", c=64),
                    )
            def emit_score(qj, hc, ki):
                b = 128 * max(0, ki - 4 * qj)
                kis = slice(128 * ki, 128 * (ki + 1))
                sc = scpool.tile([128, 2, 512], F32, tag="sc")
                nc.tensor.matmul(
                    sc[:, 0, b:512],
                    kt_sb[:, hc, kis],
                    qtz[:, 2 * hc, 512 * qj + b : 512 * (qj + 1)],
                    start=True,
                    stop=True,
                )
                nc.tensor.matmul(
                    sc[:, 1, b:512],
                    kt_sb[:, hc, kis],
                    qtz[:, 2 * hc + 1, 512 * qj + b : 512 * (qj + 1)],
                    start=True,
                    stop=True,
                )
                return sc

            steps = []
            for qj in range(2):
                kmax = 4 if qj == 0 else 8
                for hc in range(4):
                    for ki in range(kmax):
                        steps.append((qj, hc, ki, kmax))

            sc_next = emit_score(*steps[0][:3])
            avs = {}
            for i, (qj, hc, ki, kmax) in enumerate(steps):
                qsl = slice(512 * qj, 512 * (qj + 1))
                if ki == 0:
                    av_e = avpool.tile([128, 512], F32, tag="av")
                    av_o = avpool.tile([128, 512], F32, tag="av")
                    avs[(qj, hc)] = (av_e, av_o)
                o_e, o_o = avs[(qj, hc)]
                sc = sc_next
                if i + 1 < len(steps):  # cross-pair score lookahead
                    sc_next = emit_score(*steps[i + 1][:3])
                b = 128 * max(0, ki - 4 * qj)
                ee = epool.tile([128, 2, 512], BF16, tag="e")
                nc.scalar.activation(
                    ee[:, :, b:512],
                    sc[:, :, b:512],
                    EXP,
                    scale=float(SCALE),
                )
                if ki - 4 * qj >= 0:  # diagonal block: 0/1 mask
                    nc.vector.tensor_mul(
                        ee[:, 0, b : b + 128], ee[:, 0, b : b + 128], maskd
                    )
                    nc.vector.tensor_mul(
                        ee[:, 1, b : b + 128], ee[:, 1, b : b + 128], maskd
                    )
                nc.tensor.matmul(
                    o_e[0:65, b:512],
                    v_sb[:, ki, 2 * hc, :],
                    ee[:, 0, b:512],
                    start=(ki == 0),
                    stop=(ki == kmax - 1),
                )
                nc.tensor.matmul(
                    o_o[0:65, b:512],
                    v_sb[:, ki, 2 * hc + 1, :],
                    ee[:, 1, b:512],
                    start=(ki == 0),
                    stop=(ki == kmax - 1),
                )
                if ki != kmax - 1:
                    continue
                del avs[(qj, hc)]
                # extract headout^T + denominator rows
                nc.vector.tensor_copy(hout_sb[0:64, hc, qsl], o_e[0:64, :])
                otmp = xtr.tile([64, 512], BF16, tag="otmp")
                nc.vector.tensor_copy(otmp, o_o[0:64, :])
                nc.sync.dma_start(out=hout_sb[64:128, hc, qsl], in_=otmp)
                de_t = xtr.tile([1, 512], F32, tag="de")
                do_t = xtr.tile([1, 512], F32, tag="do")
                nc.vector.tensor_copy(de_t, o_e[64:65, :])
                nc.vector.tensor_copy(do_t, o_o[64:65, :])
                nc.sync.dma_start(out=den8[2 * hc : 2 * hc + 1, qsl], in_=de_t)
                nc.sync.dma_start(
                    out=den8[2 * hc + 1 : 2 * hc + 2, qsl], in_=do_t
                )
            for qj in range(2) if "O" in phases else []:
                qsl = slice(512 * qj, 512 * (qj + 1))
                # ---- normalize + output projection for this q-half
                with nc.allow_low_precision(reason="softmax reciprocal"):
                    nc.vector.reciprocal(rec8[:, qsl], den8[:, qsl])
                for hc in range(4):
                    bp = opool.tile([128, 512], F32, tag="op")
                    nc.tensor.matmul(
                        bp,
                        sel8[:, 128 * hc : 128 * (hc + 1)],
                        rec8[:, qsl],
                        start=True,
                        stop=True,
                    )
                    nc.vector.tensor_mul(
                        hout_sb[:, hc, qsl], hout_sb[:, hc, qsl], bp
                    )
                for stile in range(4 * qj, 4 * qj + 4):
                    out_sb = osb.tile([128, S], F32, tag="out")
                    for ej in range(2):
                        fp = opool.tile([128, 512], F32, tag="op")
                        for hc in range(4):
                            nc.tensor.matmul(
                                fp,
                                hout_sb[:, hc, 128 * stile : 128 * (stile + 1)],
                                wo_sb[:, hc, 512 * ej : 512 * (ej + 1)],
                                start=(hc == 0),
                                stop=(hc == 3),
                            )
                        esl = slice(512 * ej, 512 * (ej + 1))
                        if ej == 0:
                            nc.vector.tensor_copy(out_sb[:, esl], fp)
                        else:
                            nc.scalar.copy(out_sb[:, esl], fp)
                    nc.sync.dma_start(
                        out=t["out_p"][128 * stile : 128 * (stile + 1), :],
                        in_=out_sb,
                    )


def _build_phases(phases, repeat=1):
    return _build(repeat, phases=phases)


def _build(repeat=1, phases=("P", "A", "O")):
    nc = bacc.Bacc()
    t = {}
    for name in ("xq_t", "xk_t", "xv_t"):
        t[name] = nc.dram_tensor(name, [D, S], BF16, kind="ExternalInput")
    for name in ("wq_t", "wk_t", "wv_t"):
        t[name] = nc.dram_tensor(name, [128, 8 * 512], BF16, kind="ExternalInput")
    t["wo_s"] = nc.dram_tensor("wo_s", [128, 4 * D], BF16, kind="ExternalInput")
    t["maskd"] = nc.dram_tensor("maskd", [128, 128], BF16, kind="ExternalInput")
    t["sel8"] = nc.dram_tensor("sel8", [8, 512], F32R, kind="ExternalInput")
    t["ones_col"] = nc.dram_tensor("ones_col", [128, 64], BF16, kind="ExternalInput")
    t["out_p"] = nc.dram_tensor("out_p", [S, D], F32, kind="ExternalOutput")

    with tile.TileContext(nc) as tc:
        if repeat == 1:
            _emit(nc, tc, t, 0, phases)
        else:
            with tc.For_i(0, repeat, 1):
                _emit(nc, tc, t, 0, phases)
    nc.compile()
    return nc


_CACHE = {}


def _get(repeat=1):
    if repeat not in _CACHE:
        _CACHE[repeat] = _build(repeat)
    return _CACHE[repeat]


def _host_prep(query, key, value, mask, Wq, Wk, Wv, Wo):
    """Build the per-core in_maps. Returns None if mask isn't causal tril."""
    m = np.asarray(mask)[0, 0]
    if not np.array_equal(m, np.tril(np.ones((S, S), m.dtype))):
        return None

    bf = ml_dtypes.bfloat16

    # diagonal-block mask (same for every diagonal tile under causal tril)
    maskd = m[0:128, 0:128].T.astype(bf)

    sel8 = np.zeros((8, 512), np.float32)
    for hc in range(4):
        sel8[2 * hc, 128 * hc : 128 * hc + 64] = 1.0
        sel8[2 * hc + 1, 128 * hc + 64 : 128 * hc + 128] = 1.0
    ones_col = np.ones((128, 64), bf)

    def ileave(a):  # [R, C] -> [128, (R//128)*C]: chunk-c data contiguous per p
        R, C = a.shape
        return np.ascontiguousarray(
            a.reshape(R // 128, 128, C).transpose(1, 0, 2).reshape(128, -1)
        )

    in_maps = []
    for c in range(N_CORES):
        b, g = c // 2, c % 2
        gsl = slice(512 * g, 512 * (g + 1))
        in_maps.append(
            {
                "xq_t": np.ascontiguousarray(query[b].T.astype(bf)),
                "xk_t": np.ascontiguousarray(key[b].T.astype(bf)),
                "xv_t": np.ascontiguousarray(value[b].T.astype(bf)),
                "wq_t": ileave(Wq[gsl, :].T.astype(bf)),
                "wk_t": ileave(Wk[gsl, :].T.astype(bf)),
                "wv_t": ileave(Wv[gsl, :].T.astype(bf)),
                "wo_s": ileave(Wo[:, gsl].T.astype(bf)),
                "maskd": maskd,
                "sel8": sel8,
                "ones_col": ones_col,
            }
        )
    return in_maps


def _gather(results, bo, B):
    out = np.empty((B, S, D), np.float32)
    for b in range(B):
        out[b] = (
            results[2 * b]["out_p"]
            + results[2 * b + 1]["out_p"]
            + np.asarray(bo)[None, :]
        )
    return out


def _reference_fallback(query, key, value, mask, Wq, Wk, Wv, Wo, bo):
    B = query.shape[0]
    H = 16
    dk = D // H
    q = np.asarray(query, np.float32)
    k = np.asarray(key, np.float32)
    v = np.asarray(value, np.float32)

    def proj(x, W):
        return (x @ W.T).reshape(B, S, H, dk).transpose(0, 2, 1, 3)

    Q, K, V = proj(q, Wq), proj(k, Wk), proj(v, Wv)
    sc = np.einsum("bhqd,bhkd->bhqk", Q, K) / np.sqrt(np.float32(dk))
    sc = np.where(np.asarray(mask) == 0, np.float32(-1e9), sc)
    sc = sc - sc.max(axis=-1, keepdims=True)
    a = np.exp(sc)
    a = a / a.sum(axis=-1, keepdims=True)
    o = np.einsum("bhqk,bhkd->bhqd", a, V).transpose(0, 2, 1, 3).reshape(B, S, D)
    return (o @ np.asarray(Wo).T + np.asarray(bo)).astype(np.float32)


def kernel(query, key, value, mask, Wq, Wk, Wv, Wo, bo):
    query = np.asarray(query, np.float32)
    key = np.asarray(key, np.float32)
    value = np.asarray(value, np.float32)
    Wq, Wk, Wv, Wo = (np.asarray(w, np.float32) for w in (Wq, Wk, Wv, Wo))
    in_maps = _host_prep(query, key, value, mask, Wq, Wk, Wv, Wo)
    if in_maps is None:  # non-causal mask: host fallback
        return _reference_fallback(query, key, value, mask, Wq, Wk, Wv, Wo, bo)
    nc = _get(1)
    res = run_bass_kernel_spmd(nc, in_maps, list(range(N_CORES)))
    return _gather(res.results, bo, query.shape[0])


def run_spmd(in_maps, repeat=1):
    """For test.py: run prebuilt kernel, return BassKernelResults."""
    nc = _get(repeat)
    return run_bass_kernel_spmd(nc, in_maps, list(range(N_CORES)))


def host_prep(*args, **kw):
    return _host_prep(*args, **kw)


def gather(results, bo, B=4):
    return _gather(results, bo, B)


# revision 28
# speedup vs baseline: 1.0617x; 1.0245x over previous
"""Multi-head causal attention on 8 Trainium2 NeuronCores (Bass/Tile).

Problem: B=4, S=1024, D=1024, H=16 heads (dk=64), causal mask, fp32 I/O.

Sharding: 8 cores = 4 batches x 2 head-groups (8 heads each).
  Wq/Wk/Wv sharded column-wise by head (tensor parallel), Wo row-wise;
  the Wo all-reduce is a host-side pairwise sum (2 cores per batch).

Per-core kernel (bf16 matmul operands, fp32 PSUM accumulate, ~4.7e-3
absmax-relative vs the fp32 reference):
  phase P: Q^T (zero-padded per head: slot h holds Q_h^T on its 64
           partitions, zeros elsewhere, so score matmuls contract K=128
           at full rate against the packed K^T without mixing heads),
           K^T packed [128, 4, S], V -> v_sb [128, ki, head, 65] with a
           65th ones column per head (softmax denominator trick).
           PSUM->SBUF copies run on the otherwise-idle ACT engine.
  phase A: per head-chunk hc and q-half qj: scores^T [k=128, q<=512]
           (causally width-trimmed), exp on ACT (no max subtraction:
           |scores/8| < ~6), 0/1 mask multiply only on the diagonal
           128-block, attnV accumulated over k-chunks with lhsT =
           V_ext [k, 65]; row 64 = denominator. Denominator rows are
           copied on ACT and DMA-gathered into den8 (keeps the DVE FIFO
           and PE stream free of per-pair round-trips).
  phase O: two batched reciprocals, selector-matmul broadcast of 1/den
           over partition halves, in-place normalize of headout^T,
           output projection accumulating over d-chunks, DMA out.
"""

from contextlib import ExitStack

import ml_dtypes
import numpy as np

import concourse.bacc as bacc
import concourse.tile as tile
from concourse import mybir
from concourse.bass_utils import run_bass_kernel_spmd

A_SUB = {"mask", "av", "extract", "houtcp", "recip"}  # phase-A bisection knob

F32R = mybir.dt.float32r
F32 = mybir.dt.float32
BF16 = mybir.dt.bfloat16
EXP = mybir.ActivationFunctionType.Exp

S = 1024  # sequence length
D = 1024  # model dim
DK = 64  # head dim
HPC = 8  # heads per core
N_CORES = 8
SCALE = 1.0 / np.sqrt(DK)  # folded into the exp activation


def _emit(nc, tc, t, rep, phases=("P", "A", "O")):
    """Emit one full forward pass. `t` = dict of dram tensors."""
    ctx = ExitStack()
    with ctx:
        # ---- long-lived SBUF (per repeat; pools free at phase end) ----
        main = ctx.enter_context(tc.tile_pool(name=f"main{rep}", bufs=1))
        xpool = ctx.enter_context(tc.tile_pool(name=f"xin{rep}", bufs=2))
        wpool = ctx.enter_context(tc.tile_pool(name=f"win{rep}", bufs=2))

        # Q^T zero-padded per head: slot h holds Q_h^T on its 64 partitions,
        # zeros on the other 64 -> score matmuls contract K=128 (full rate)
        # against the packed kt_sb without mixing heads.
        qtz = main.tile([128, 8, S], BF16)
        kt_sb = main.tile([128, 4, S], BF16)
        v_sb = main.tile([128, 8, 8, 65], BF16)  # s-part: (ki, head, d+1)
        hout_sb = main.tile([128, 4, S], BF16)  # headout^T (unnormalized)
        maskd = main.tile([128, 128], BF16)  # diagonal-block 0/1 mask
        sel8 = main.tile([8, 512], F32R)
        den8 = main.tile([8, S], F32)
        rec8 = main.tile([8, S], F32R)
        wo_sb = main.tile([128, 4, S], BF16)

        nc.sync.dma_start(out=sel8, in_=t["sel8"][:, :])
        nc.sync.dma_start(out=maskd, in_=t["maskd"][:, :])
        nc.sync.dma_start(
            out=v_sb.rearrange("p a b c -> p (a b) c")[:, :, 64:65],
            in_=t["ones_col"][:, :, None],
        )
        nc.vector.memset(qtz.rearrange("p a b -> p (a b)"), 0.0)

        # ================= phase P: projections =================
        if "P" in phases:
         with (
            tc.tile_pool(name=f"pps{rep}", bufs=2, space="PSUM") as ppool,
        ):
            for which, xname, wname in (
                ("q", "xq_t", "wq_t"),
                ("k", "xk_t", "wk_t"),
            ):
                x_sb = xpool.tile([128, 8, S], BF16, tag="x")
                w_sb = wpool.tile([128, 8, 512], BF16, tag="w")
                xdr = t[xname].rearrange("(n p) s -> p n s", p=128)
                nc.sync.dma_start(
                    out=w_sb.rearrange("p a b -> p (a b)"), in_=t[wname][:, :]
                )
                for half in range(2):  # column halves: s-half chains start early
                    nc.sync.dma_start(
                        out=x_sb[:, :, 512 * half : 512 * (half + 1)],
                        in_=xdr[:, :, 512 * half : 512 * (half + 1)],
                    )
                if True:
                    for sj in range(2):
                        for dtile in range(4):
                            ps = ppool.tile([128, 512], F32, tag="ps")
                            for c in range(8):
                                nc.tensor.matmul(
                                    ps,
                                    w_sb[:, c, 128 * dtile : 128 * (dtile + 1)],
                                    x_sb[:, c, 512 * sj : 512 * (sj + 1)],
                                    start=(c == 0),
                                    stop=(c == 7),
                                )
                            sjs = slice(512 * sj, 512 * (sj + 1))
                            if which == "q":
                                nc.scalar.copy(
                                    qtz[0:64, 2 * dtile, sjs], ps[0:64, :]
                                )
                                nc.scalar.copy(
                                    qtz[64:128, 2 * dtile + 1, sjs], ps[64:128, :]
                                )
                            else:
                                nc.scalar.copy(kt_sb[:, dtile, sjs], ps)

        # ========= phase A + O fused: per q-half, attention then outproj ====
        if "A" in phases:
         with (
            tc.tile_pool(name=f"avps{rep}", bufs=2, space="PSUM") as avpool,
            tc.tile_pool(name=f"epool{rep}", bufs=12) as epool,
            tc.tile_pool(name=f"xtr{rep}", bufs=6) as xtr,
            tc.tile_pool(name=f"osb{rep}", bufs=3) as osb,
        ):
            if "O" in phases:  # prefetch Wo during attention
                nc.sync.dma_start(
                    out=wo_sb.rearrange("p a b -> p (a b)"), in_=t["wo_s"][:, :]
                )
            scpool = ctx.enter_context(
                tc.tile_pool(name=f"scps{rep}", bufs=2, space="PSUM")
            ) if False else None
            scpool_cm = tc.tile_pool(name=f"scps{rep}", bufs=2, space="PSUM")
            scpool = scpool_cm.__enter__()
            # V projection shares the score pool so Q.K scores/exp overlap it
            xv_sb = xpool.tile([128, 8, S], BF16, tag="x")
            wv_sb = wpool.tile([128, 8, 512], BF16, tag="w")
            xvdr = t["xv_t"].rearrange("(n p) s -> p n s", p=128)
            nc.sync.dma_start(
                out=wv_sb.rearrange("p a b -> p (a b)"), in_=t["wv_t"][:, :]
            )
            for half in range(2):
                nc.sync.dma_start(
                    out=xv_sb[:, :, 512 * half : 512 * (half + 1)],
                    in_=xvdr[:, :, 512 * half : 512 * (half + 1)],
                )
            for tpair in range(4):
                ps2 = scpool.tile([128, 2, 512], F32, tag="sc")
                for sub in range(2):
                    stile = 2 * tpair + sub
                    for c in range(8):
                        nc.tensor.matmul(
                            ps2[:, sub, :],
                            xv_sb[:, c, 128 * stile : 128 * (stile + 1)],
                            wv_sb[:, c, :],
                            start=(c == 0),
                            stop=(c == 7),
                        )
                    nc.scalar.copy(
                        v_sb[:, stile, :, 0:64],
                        ps2[:, sub, :].rearrange("p (h c) -> p h c", c=64),
                    )
            def emit_score(qj, hc, ki):
                b = 128 * max(0, ki - 4 * qj)
                kis = slice(128 * ki, 128 * (ki + 1))
                sc = scpool.tile([128, 2, 512], F32, tag="sc")
                nc.tensor.matmul(
                    sc[:, 0, b:512],
                    kt_sb[:, hc, kis],
                    qtz[:, 2 * hc, 512 * qj + b : 512 * (qj + 1)],
                    start=True,
                    stop=True,
                )
                nc.tensor.matmul(
                    sc[:, 1, b:512],
                    kt_sb[:, hc, kis],
                    qtz[:, 2 * hc + 1, 512 * qj + b : 512 * (qj + 1)],
                    start=True,
                    stop=True,
                )
                return sc

            steps = []
            for qj in range(2):
                kmax = 4 if qj == 0 else 8
                for hc in range(4):
                    for ki in range(kmax):
                        steps.append((qj, hc, ki, kmax))

            sc_next = emit_score(*steps[0][:3])
            avs = {}
            for i, (qj, hc, ki, kmax) in enumerate(steps):
                qsl = slice(512 * qj, 512 * (qj + 1))
                if ki == 0:
                    av_e = avpool.tile([128, 512], F32, tag="av")
                    av_o = avpool.tile([128, 512], F32, tag="av")
                    avs[(qj, hc)] = (av_e, av_o)
                o_e, o_o = avs[(qj, hc)]
                sc = sc_next
                if i + 1 < len(steps):  # cross-pair score lookahead
                    sc_next = emit_score(*steps[i + 1][:3])
                b = 128 * max(0, ki - 4 * qj)
                ee = epool.tile([128, 2, 512], BF16, tag="e")
                nc.scalar.activation(
                    ee[:, :, b:512],
                    sc[:, :, b:512],
                    EXP,
                    scale=float(SCALE),
                )
                if ki - 4 * qj >= 0:  # diagonal block: 0/1 mask
                    nc.vector.tensor_mul(
                        ee[:, 0, b : b + 128], ee[:, 0, b : b + 128], maskd
                    )
                    nc.vector.tensor_mul(
                        ee[:, 1, b : b + 128], ee[:, 1, b : b + 128], maskd
                    )
                nc.tensor.matmul(
                    o_e[0:65, b:512],
                    v_sb[:, ki, 2 * hc, :],
                    ee[:, 0, b:512],
                    start=(ki == 0),
                    stop=(ki == kmax - 1),
                )
                nc.tensor.matmul(
                    o_o[0:65, b:512],
                    v_sb[:, ki, 2 * hc + 1, :],
                    ee[:, 1, b:512],
                    start=(ki == 0),
                    stop=(ki == kmax - 1),
                )
                if ki != kmax - 1:
                    continue
                del avs[(qj, hc)]
                # extract headout^T + denominator rows
                nc.vector.tensor_copy(hout_sb[0:64, hc, qsl], o_e[0:64, :])
                otmp = xtr.tile([64, 512], BF16, tag="otmp")
                nc.vector.tensor_copy(otmp, o_o[0:64, :])
                nc.sync.dma_start(out=hout_sb[64:128, hc, qsl], in_=otmp)
                de_t = xtr.tile([1, 512], F32, tag="de")
                do_t = xtr.tile([1, 512], F32, tag="do")
                nc.vector.tensor_copy(de_t, o_e[64:65, :])
                nc.vector.tensor_copy(do_t, o_o[64:65, :])
                nc.sync.dma_start(out=den8[2 * hc : 2 * hc + 1, qsl], in_=de_t)
                nc.sync.dma_start(
                    out=den8[2 * hc + 1 : 2 * hc + 2, qsl], in_=do_t
                )
            scpool_cm.__exit__(None, None, None)
            opool_cm = tc.tile_pool(name=f"ops{rep}", bufs=4, space="PSUM")
            opool = opool_cm.__enter__()
            for qj in range(2) if "O" in phases else []:
                qsl = slice(512 * qj, 512 * (qj + 1))
                # ---- normalize + output projection for this q-half
                with nc.allow_low_precision(reason="softmax reciprocal"):
                    nc.vector.reciprocal(rec8[:, qsl], den8[:, qsl])
                for hc in range(4):
                    bp = avpool.tile([128, 512], F32, tag="bp")
                    nc.tensor.matmul(
                        bp,
                        sel8[:, 128 * hc : 128 * (hc + 1)],
                        rec8[:, qsl],
                        start=True,
                        stop=True,
                    )
                    nc.vector.tensor_mul(
                        hout_sb[:, hc, qsl], hout_sb[:, hc, qsl], bp
                    )
                for stile in range(4 * qj, 4 * qj + 4):
                    out_sb = osb.tile([128, S], F32, tag="out")
                    for ej in range(2):
                        fp = opool.tile([128, 512], F32, tag="op")
                        for hc in range(4):
                            nc.tensor.matmul(
                                fp,
                                hout_sb[:, hc, 128 * stile : 128 * (stile + 1)],
                                wo_sb[:, hc, 512 * ej : 512 * (ej + 1)],
                                start=(hc == 0),
                                stop=(hc == 3),
                            )
                        esl = slice(512 * ej, 512 * (ej + 1))
                        if ej == 0:
                            nc.vector.tensor_copy(out_sb[:, esl], fp)
                        else:
                            nc.scalar.copy(out_sb[:, esl], fp)
                    nc.sync.dma_start(
                        out=t["out_p"][128 * stile : 128 * (stile + 1), :],
                        in_=out_sb,
                    )
            opool_cm.__exit__(None, None, None)


def _build_phases(phases, repeat=1):
    return _build(repeat, phases=phases)


def _build(repeat=1, phases=("P", "A", "O")):
    nc = bacc.Bacc()
    t = {}
    for name in ("xq_t", "xk_t", "xv_t"):
        t[name] = nc.dram_tensor(name, [D, S], BF16, kind="ExternalInput")
    for name in ("wq_t", "wk_t", "wv_t"):
        t[name] = nc.dram_tensor(name, [128, 8 * 512], BF16, kind="ExternalInput")
    t["wo_s"] = nc.dram_tensor("wo_s", [128, 4 * D], BF16, kind="ExternalInput")
    t["maskd"] = nc.dram_tensor("maskd", [128, 128], BF16, kind="ExternalInput")
    t["sel8"] = nc.dram_tensor("sel8", [8, 512], F32R, kind="ExternalInput")
    t["ones_col"] = nc.dram_tensor("ones_col", [128, 64], BF16, kind="ExternalInput")
    t["out_p"] = nc.dram_tensor("out_p", [S, D], F32, kind="ExternalOutput")

    with tile.TileContext(nc) as tc:
        if repeat == 1:
            _emit(nc, tc, t, 0, phases)
        else:
            with tc.For_i(0, repeat, 1):
                _emit(nc, tc, t, 0, phases)
    nc.compile()
    return nc


_CACHE = {}


def _get(repeat=1):
    if repeat not in _CACHE:
        _CACHE[repeat] = _build(repeat)
    return _CACHE[repeat]


def _host_prep(query, key, value, mask, Wq, Wk, Wv, Wo):
    """Build the per-core in_maps. Returns None if mask isn't causal tril."""
    m = np.asarray(mask)[0, 0]
    if not np.array_equal(m, np.tril(np.ones((S, S), m.dtype))):
        return None

    bf = ml_dtypes.bfloat16

    # diagonal-block mask (same for every diagonal tile under causal tril)
    maskd = m[0:128, 0:128].T.astype(bf)

    sel8 = np.zeros((8, 512), np.float32)
    for hc in range(4):
        sel8[2 * hc, 128 * hc : 128 * hc + 64] = 1.0
        sel8[2 * hc + 1, 128 * hc + 64 : 128 * hc + 128] = 1.0
    ones_col = np.ones((128, 64), bf)

    def ileave(a):  # [R, C] -> [128, (R//128)*C]: chunk-c data contiguous per p
        R, C = a.shape
        return np.ascontiguousarray(
            a.reshape(R // 128, 128, C).transpose(1, 0, 2).reshape(128, -1)
        )

    in_maps = []
    for c in range(N_CORES):
        b, g = c // 2, c % 2
        gsl = slice(512 * g, 512 * (g + 1))
        in_maps.append(
            {
                "xq_t": np.ascontiguousarray(query[b].T.astype(bf)),
                "xk_t": np.ascontiguousarray(key[b].T.astype(bf)),
                "xv_t": np.ascontiguousarray(value[b].T.astype(bf)),
                "wq_t": ileave(Wq[gsl, :].T.astype(bf)),
                "wk_t": ileave(Wk[gsl, :].T.astype(bf)),
                "wv_t": ileave(Wv[gsl, :].T.astype(bf)),
                "wo_s": ileave(Wo[:, gsl].T.astype(bf)),
                "maskd": maskd,
                "sel8": sel8,
                "ones_col": ones_col,
            }
        )
    return in_maps


def _gather(results, bo, B):
    out = np.empty((B, S, D), np.float32)
    for b in range(B):
        out[b] = (
            results[2 * b]["out_p"]
            + results[2 * b + 1]["out_p"]
            + np.asarray(bo)[None, :]
        )
    return out


def _reference_fallback(query, key, value, mask, Wq, Wk, Wv, Wo, bo):
    B = query.shape[0]
    H = 16
    dk = D // H
    q = np.asarray(query, np.float32)
    k = np.asarray(key, np.float32)
    v = np.asarray(value, np.float32)

    def proj(x, W):
        return (x @ W.T).reshape(B, S, H, dk).transpose(0, 2, 1, 3)

    Q, K, V = proj(q, Wq), proj(k, Wk), proj(v, Wv)
    sc = np.einsum("bhqd,bhkd->bhqk", Q, K) / np.sqrt(np.float32(dk))
    sc = np.where(np.asarray(mask) == 0, np.float32(-1e9), sc)
    sc = sc - sc.max(axis=-1, keepdims=True)
    a = np.exp(sc)
    a = a / a.sum(axis=-1, keepdims=True)
    o = np.einsum("bhqk,bhkd->bhqd", a, V).transpose(0, 2, 1, 3).reshape(B, S, D)
    return (o @ np.asarray(Wo).T + np.asarray(bo)).astype(np.float32)


def kernel(query, key, value, mask, Wq, Wk, Wv, Wo, bo):
    query = np.asarray(query, np.float32)
    key = np.asarray(key, np.float32)
    value = np.asarray(value, np.float32)
    Wq, Wk, Wv, Wo = (np.asarray(w, np.float32) for w in (Wq, Wk, Wv, Wo))
    in_maps = _host_prep(query, key, value, mask, Wq, Wk, Wv, Wo)
    if in_maps is None:  # non-causal mask: host fallback
        return _reference_fallback(query, key, value, mask, Wq, Wk, Wv, Wo, bo)
    nc = _get(1)
    res = run_bass_kernel_spmd(nc, in_maps, list(range(N_CORES)))
    return _gather(res.results, bo, query.shape[0])


def run_spmd(in_maps, repeat=1):
    """For test.py: run prebuilt kernel, return BassKernelResults."""
    nc = _get(repeat)
    return run_bass_kernel_spmd(nc, in_maps, list(range(N_CORES)))


def host_prep(*args, **kw):
    return _host_prep(*args, **kw)


def gather(results, bo, B=4):
    return _gather(results, bo, B)
